# revision 17
# baseline (speedup 1.0000x reference)
"""Trainium2 Bass kernel for the GNN k-hop subgraph encoder (GIN, L=2, D=256).

v2 strategy (8 cores, graph-parallel, flat per-core layout):
  - Host: sort subgraph nodes by indicator; core c owns centers
    [c*2500,(c+1)*2500) and packs its subgraph rows flat (NR = max rows,
    padded to 512). Host builds type-count matrices, int16 gather indices
    (wrapped [16,n/16] for the Q7 cores), dst one-hot offsets, pooling
    offsets, and folds the embedding tables into W1 (TAB1W1 = TAB1@W1[0],
    EW1 = E2@W1[1]) so layer-1 aggregation is a single 19-contraction
    matmul off the count matrix.
  - Layer 1: mid = TAB1W1^T @ cnt19 per 512-node chunk, h1 = relu(... W2).
    h1 chunks stream to DRAM; a 5-way chunked AllGather (Shared output)
    overlaps the collective with remaining layer-1 compute.
  - Layer 2 per chunk: batched dma_gather (one per src region, int16 idx)
    pulls all neighbor h1 rows; dst one-hots are built on-device
    (iota + is_equal) and scatter-added via matmuls into a [128,512] PSUM
    per feature half; self-loops are identity matmuls on re-read h1 rows.
    The GIN MLP follows with edge-type contributions folded via EW1.
  - Pooling is eager per chunk: one-hot matmuls into a 512-center window,
    vector-added into the cat accumulator.
  - BN stats (2x) via tiny AllReduce; projection + final norm + transpose
    on device. Host concatenates the 8 output slices.
"""
import os
import sys

import numpy as np

sys.path.insert(0, "/opt/trn_rl_repo")

N = 20000
NSUB = 100000
D = 256
NCORE = 8
CPC = N // NCORE            # 2500 centers per core
CPAD = 2560
NBLK = CPAD // 128          # 20
EPS = 1e-5
NREG = 5
PIECE = 2560


# ----------------------------------------------------------------------------
# host preprocessing
# ----------------------------------------------------------------------------
def _preprocess(inputs):
    x = np.asarray(inputs["x"], np.int64)
    sni = np.asarray(inputs["subgraph_node_index"], np.int64)
    sei = np.asarray(inputs["subgraph_edge_index"], np.int64)
    sea = np.asarray(inputs["subgraph_edge_attr"], np.int64)
    sii = np.asarray(inputs["subgraph_indicator_index"], np.int64)

    pi = np.argsort(sii, kind="stable")
    inv = np.empty(NSUB, np.int64)
    inv[pi] = np.arange(NSUB)
    ind_s = sii[pi]
    node_s = sni[pi]

    sub_lo = np.searchsorted(ind_s, np.arange(0, N + 1, CPC))
    rc = np.diff(sub_lo)
    NR = int(np.ceil(rc.max() / 512) * 512)
    NCH = NR // 512
    NT = NR // 128

    core_of_sorted = np.searchsorted(sub_lo, np.arange(NSUB), side="right") - 1
    flat_sorted = np.arange(NSUB) - sub_lo[core_of_sorted]
    core_of = core_of_sorted[inv]
    loc_of = flat_sorted[inv]
    gslot = core_of * NR + loc_of

    ecombo = sea[:, 0] * 3 + sea[:, 1]
    ntype_sorted = x[node_s, 0] * 3 + x[node_s, 1]
    ntype_of = ntype_sorted[inv]

    cnt19 = np.zeros((NCORE, 19, NR), np.float16)
    e_src, e_dst = sei[0], sei[1]
    dc, dl = core_of[e_dst], loc_of[e_dst]
    np.add.at(cnt19, (dc, ntype_of[e_src], dl), 1.0)
    np.add.at(cnt19, (dc, 9 + ecombo, dl), 1.0)
    for c in range(NCORE):
        rows = np.arange(rc[c])
        srt = slice(sub_lo[c], sub_lo[c + 1])
        np.add.at(cnt19[c], (ntype_sorted[srt], rows), 1.0)
        cnt19[c, 18, rows] += 1.0

    # layer-2 scatter groups per (core, chunk, AG piece), sorted bucketing
    assert NR % PIECE == 0 and NR // PIECE == NREG
    reg_all = loc_of // PIECE
    rel_all = core_of * PIECE + (loc_of % PIECE)
    src_rel = rel_all[e_src]
    dst_rel = dl - (dl // 512) * 512
    key = ((dc * NCH + dl // 512) * NREG + reg_all[e_src]).astype(np.int64)
    order = np.argsort(key, kind="stable")
    ks, ss, ds = key[order], src_rel[order], dst_rel[order]
    nkey = NCORE * NCH * NREG
    bounds = np.searchsorted(ks, np.arange(nkey + 1))
    cnt_kg = np.diff(bounds).reshape(NCORE, NCH, NREG)
    TPG = np.ceil(cnt_kg.max(axis=0) / 128).astype(np.int64)  # [NCH, NREG]
    T_E = int(TPG.sum())

    idx16 = np.zeros((NCORE, 128, T_E * 8), np.int16)
    dsto = np.full((NCORE, 128, T_E), 1000.0, np.float32)
    for c in range(NCORE):
        t0 = 0
        for ch in range(NCH):
            for g in range(NREG):
                k = (c * NCH + ch) * NREG + g
                a, b = bounds[k], bounds[k + 1]
                npad = int(TPG[ch, g]) * 128
                sv = np.zeros(npad, np.int64)
                sv[: b - a] = ss[a:b]
                a16 = sv.reshape(npad // 16, 16).T
                idx16[c, :, t0 * 8: t0 * 8 + npad // 16] = np.tile(a16, (8, 1))
                dv = np.full(npad, 1000.0)
                dv[: b - a] = ds[a:b]
                dsto[c, :, t0: t0 + int(TPG[ch, g])] = dv.reshape(-1, 128).T
                t0 += int(TPG[ch, g])
        assert t0 == T_E

    # pooling offsets: per node tile, center col within the chunk's window
    pool_off = np.full((NCORE, 128, NT), 1000.0, np.float32)
    pbase = np.zeros((NCH,), np.int64)
    for ch in range(NCH):
        firsts, lasts = [], []
        for c in range(NCORE):
            lo, hi = ch * 512, min((ch + 1) * 512, int(rc[c]))
            if lo >= rc[c]:
                continue
            firsts.append(int(ind_s[sub_lo[c] + lo]) - c * CPC)
            lasts.append(int(ind_s[sub_lo[c] + hi - 1]) - c * CPC)
        b = min(firsts)
        assert max(lasts) - b < 512, (ch, b, max(lasts))
        pbase[ch] = min(b, CPAD - 512)
    for c in range(NCORE):
        for ch in range(NCH):
            lo, hi = ch * 512, min((ch + 1) * 512, int(rc[c]))
            if lo >= rc[c]:
                continue
            srt = slice(sub_lo[c] + lo, sub_lo[c] + hi)
            offs = ind_s[srt] - c * CPC - pbase[ch]
            rows = np.arange(lo, hi)
            pool_off[c, rows % 128, rows // 128] = offs.astype(np.float32)

    oh9 = np.zeros((NCORE, 9, CPAD), np.float16)
    for c in range(NCORE):
        cn = np.arange(c * CPC, (c + 1) * CPC)
        oh9[c, x[cn, 0] * 3 + x[cn, 1], np.arange(CPC)] = 1.0

    per_core = [
        dict(cnt19=cnt19[c], idx16=idx16[c], dsto=dsto[c],
             pool_off=pool_off[c], oh9=oh9[c])
        for c in range(NCORE)
    ]
    meta = dict(NR=NR, NCH=NCH, NT=NT, T_E=T_E,
                TPG=[[int(v) for v in row] for row in TPG],
                pbase=[int(v) for v in pbase])
    return per_core, meta


def _weight_maps(inputs):
    f16 = np.float16
    f32 = np.float32
    emb1 = np.asarray(inputs["emb1"], f32)
    emb2 = np.asarray(inputs["emb2"], f32)
    ee1 = np.asarray(inputs["edge_e1"], f32)
    ee2 = np.asarray(inputs["edge_e2"], f32)
    W1 = np.asarray(inputs["W1"], f32)
    W2 = np.asarray(inputs["W2"], f32)

    TAB1 = np.zeros((19, D), f32)
    for t in range(9):
        TAB1[t] = emb1[t // 3] + emb2[t % 3]
    for u in range(9):
        TAB1[9 + u] = ee1[0][u // 3] + ee2[0][u % 3]
    TAB1[18] = ee1[0][4] + ee2[0][0]
    E2 = np.zeros((10, D), f32)
    for u in range(9):
        E2[u] = ee1[1][u // 3] + ee2[1][u % 3]
    E2[9] = ee1[1][4] + ee2[1][0]

    return dict(
        tab1w1a=(TAB1[:9] @ W1[0]).astype(f16),      # [9, 512]
        tab1w1b=(TAB1[9:] @ W1[0]).astype(f16),      # [10, 512]
        ew1=(E2 @ W1[1]).astype(f16),                # [10, 512]
        tab9=TAB1[:9].astype(f16),                   # [9, 256]
        w1l2=W1[1].astype(f16),                      # [256, 512]
        w2=W2.astype(f16),                           # [2, 512, 256]
        b1t=np.asarray(inputs["b1"], f32).reshape(2, 4, 128, 1),
        b2f=np.asarray(inputs["b2"], f32).reshape(2, 1, 256).astype(f16),
        wp=np.asarray(inputs["Wp"], f32).astype(f16),
        bpt=np.asarray(inputs["bp"], f32).reshape(2, 128, 1),
        bngt=np.asarray(inputs["bn_cat_g"], f32).reshape(4, 128, 1),
        bnbt=np.asarray(inputs["bn_cat_b"], f32).reshape(4, 128, 1),
        ngt=np.asarray(inputs["norm_g"], f32).reshape(2, 128, 1),
        nbt=np.asarray(inputs["norm_b"], f32).reshape(2, 128, 1),
    )


# ----------------------------------------------------------------------------
# bass kernel
# ----------------------------------------------------------------------------
def _build(meta):
    from concourse import bass, bacc, mybir, tile
    from concourse.masks import make_identity

    f16 = mybir.dt.float16
    f32 = mybir.dt.float32
    i16 = mybir.dt.int16
    i32 = mybir.dt.int32
    AF = mybir.ActivationFunctionType
    OP = mybir.AluOpType

    NR = meta["NR"]
    NCH = meta["NCH"]
    NT = meta["NT"]
    T_E = meta["T_E"]
    TPG = meta["TPG"]
    PBASE = meta["pbase"]
    AGCH = NREG                    # AllGather pieces = gather regions
    AGR = PIECE                    # rows per piece (2560)
    assert NR == AGCH * AGR and AGR % 512 == 0

    nc = bacc.Bacc("TRN2", target_bir_lowering=False, debug=False,
                   num_devices=NCORE)

    def din(name, shape, dt):
        return nc.dram_tensor(name, shape, dt, kind="ExternalInput")

    cnt19 = din("cnt19", [19, NR], f16)
    _ = cnt19
    idx16 = din("idx16", [128, T_E * 8], i16)
    dsto = din("dsto", [128, T_E], f32)
    pool_off = din("pool_off", [128, NT], f32)
    oh9 = din("oh9", [9, CPAD], f16)
    tab1w1a = din("tab1w1a", [9, 512], f16)
    tab1w1b = din("tab1w1b", [10, 512], f16)
    ew1 = din("ew1", [10, 512], f16)
    tab9 = din("tab9", [9, 256], f16)
    w1l2 = din("w1l2", [256, 512], f16)
    w2 = din("w2", [2, 512, 256], f16)
    b1t = din("b1t", [2, 4, 128, 1], f32)
    b2f = din("b2f", [2, 1, 256], f16)
    wp = din("wp", [512, 256], f16)
    bpt = din("bpt", [2, 128, 1], f32)
    bngt = din("bngt", [4, 128, 1], f32)
    bnbt = din("bnbt", [4, 128, 1], f32)
    ngt = din("ngt", [2, 128, 1], f32)
    nbt = din("nbt", [2, 128, 1], f32)
    out = nc.dram_tensor("out", [CPAD, 256], f32, kind="ExternalOutput")

    TPGMAX = max(max(row) for row in TPG)

    with tile.TileContext(nc) as tc:
        with (
            tc.tile_pool(name="const", bufs=1) as cpool,
            tc.tile_pool(name="wide", bufs=1) as wide,
            tc.tile_pool(name="work", bufs=2) as work,
            tc.tile_pool(name="mids", bufs=5) as midp,
            
            tc.tile_pool(name="statp", bufs=6) as statp,
            tc.tile_pool(name="msgs", bufs=2) as msgp,
            tc.tile_pool(name="selfp", bufs=4) as selfp,
            tc.tile_pool(name="ohs", bufs=4) as ohp,
            tc.tile_pool(name="h2s", bufs=4) as h2p,
            tc.tile_pool(name="h1s", bufs=4) as h1p,
            tc.tile_pool(name="psA", bufs=4, space="PSUM") as psA,
            tc.tile_pool(name="psB", bufs=2, space="PSUM") as psB,
            tc.tile_pool(name="psC", bufs=2, space="PSUM") as psC,
            tc.tile_pool(name="dram", bufs=1, space="DRAM") as dram,
        ):
            # ---------------- constants / weights into SBUF ----------------
            _ldc = [0]

            def load(pool, src, shape, dt):
                _ldc[0] += 1
                nm = f"ld{_ldc[0]}"
                t = pool.tile(shape, dt, name=nm, tag=nm)
                nc.sync.dma_start(out=t[:], in_=src)
                return t

            cntN_sb = load(cpool, cnt19[0:9, :], [9, NR], f16)
            cntE_sb = load(cpool, cnt19[9:19, :], [10, NR], f16)
            idx_sb = load(cpool, idx16[:, :], [128, T_E * 8], i16)
            dsto_sb = load(cpool, dsto[:, :], [128, T_E], f32)
            poff_sb = load(cpool, pool_off[:, :], [128, NT], f32)
            oh9_sb = load(cpool, oh9[:, :], [9, CPAD], f16)
            tab1w1a_sb = load(cpool, tab1w1a[:, :], [9, 512], f16)
            tab1w1b_sb = load(cpool, tab1w1b[:, :], [10, 512], f16)
            ew1_sb = load(cpool, ew1[:, :], [10, 512], f16)
            tab9_sb = load(cpool, tab9[:, :], [9, 256], f16)
            w1_sb = [load(cpool, w1l2[k * 128:(k + 1) * 128, :],
                          [128, 512], f16) for k in range(2)]
            w2_sb = [[load(cpool, w2[l, k * 128:(k + 1) * 128, :],
                           [128, 256], f16) for k in range(4)]
                     for l in range(2)]
            wp_sb = [load(cpool, wp[k * 128:(k + 1) * 128, :],
                          [128, 256], f16) for k in range(4)]
            b1_sb = [[load(cpool, b1t[l, m], [128, 1], f32) for m in range(4)]
                     for l in range(2)]
            b2_sb = [load(cpool, b2f[l], [1, 256], f16) for l in range(2)]
            bp_sb = [load(cpool, bpt[c2], [128, 1], f32) for c2 in range(2)]
            bng_sb = [load(cpool, bngt[t], [128, 1], f32) for t in range(4)]
            bnb_sb = [load(cpool, bnbt[t], [128, 1], f32) for t in range(4)]
            ng_sb = [load(cpool, ngt[t], [128, 1], f32) for t in range(2)]
            nb_sb = [load(cpool, nbt[t], [128, 1], f32) for t in range(2)]

            ones_sb = cpool.tile([1, 128], f16)
            nc.vector.memset(ones_sb[:], 1.0)
            eps_sb = cpool.tile([128, 1], f32)
            nc.vector.memset(eps_sb[:], EPS)
            ident32 = cpool.tile([128, 128], f32)
            make_identity(nc, ident32[:])
            ident16 = cpool.tile([128, 128], f16)
            make_identity(nc, ident16[:])
            iota_f = cpool.tile([128, 512], f16)
            nc.gpsimd.iota(iota_f[:], pattern=[[1, 512]], base=0,
                           channel_multiplier=0,
                           allow_small_or_imprecise_dtypes=True)

            # cat accumulators (x_struct halves zeroed; origin overwritten)
            cat_sb = [wide.tile([128, CPAD], f32, tag=f"cat{t}",
                                name=f"cat{t}") for t in range(4)]
            for k in range(2):
                nc.vector.memset(cat_sb[2 + k][:], 0.0)

            # DRAM bounces
            h1loc = dram.tile([NR, 256], f16)
            h1full = [dram.tile([NCORE, AGR, 256], f16, addr_space="Shared",
                                name=f"h1full{g}", tag=f"h1full{g}")
                      for g in range(AGCH)]
            st1loc = dram.tile([512, 2], f32)
            st1glob = dram.tile([512, 2], f32, addr_space="Shared")
            st2loc = dram.tile([256, 2], f32)
            st2glob = dram.tile([256, 2], f32, addr_space="Shared")

            # ---------------- phase B: layer 1 (+ chunked AllGather) -------
            for ch in range(NCH):
                mid_sb = []
                for m in range(4):
                    mp = psB.tile([128, 512], f32, space="PSUM", tag="psB")
                    nc.tensor.matmul(
                        mp[:], lhsT=tab1w1a_sb[:, m * 128:(m + 1) * 128],
                        rhs=cntN_sb[:, ch * 512:(ch + 1) * 512],
                        start=True, stop=False)
                    nc.tensor.matmul(
                        mp[:], lhsT=tab1w1b_sb[:, m * 128:(m + 1) * 128],
                        rhs=cntE_sb[:, ch * 512:(ch + 1) * 512],
                        start=False, stop=True)
                    ms = midp.tile([128, 512], f16, tag="mid")
                    nc.scalar.activation(out=ms[:], in_=mp[:], func=AF.Relu,
                                         bias=b1_sb[0][m][:])
                    mid_sb.append(ms)
                for r in range(4):
                    hp = psC.tile([128, 256], f32, space="PSUM", tag="psC")
                    for k in range(4):
                        nc.tensor.matmul(
                            hp[:], lhsT=mid_sb[k][:, r * 128:(r + 1) * 128],
                            rhs=w2_sb[0][k][:], start=(k == 0), stop=False)
                    nc.tensor.matmul(hp[:], lhsT=ones_sb[:], rhs=b2_sb[0][:],
                                     start=False, stop=True)
                    hs = h1p.tile([128, 256], f16, tag="h1out")
                    nc.scalar.activation(out=hs[:], in_=hp[:], func=AF.Relu)
                    rt = ch * 4 + r
                    nc.sync.dma_start(out=h1loc[rt * 128:(rt + 1) * 128, :],
                                      in_=hs[:])
                # interleave AllGather pieces as their input rows complete
                if (ch + 1) % (NCH // AGCH) == 0:
                    g = (ch + 1) // (NCH // AGCH) - 1
                    nc.gpsimd.collective_compute(
                        "AllGather", OP.bypass,
                        replica_groups=[list(range(NCORE))],
                        ins=[h1loc[g * AGR:(g + 1) * AGR, :].opt()],
                        outs=[h1full[g][:, :, :].opt()])

            # origin -> cat fm (independent of AG; fills the AG window)
            for k in range(2):
                for w in range(CPAD // 512):
                    op_ = psA.tile([128, 512], f32, space="PSUM", tag="psA")
                    nc.tensor.matmul(
                        op_[:], lhsT=tab9_sb[:, k * 128:(k + 1) * 128],
                        rhs=oh9_sb[:, w * 512:(w + 1) * 512],
                        start=True, stop=True)
                    nc.vector.tensor_copy(
                        out=cat_sb[k][:, w * 512:(w + 1) * 512], in_=op_[:])

            h1flat = [h1full[g][:, :, :].flatten_outer_dims()
                      for g in range(AGCH)]

            # ---------------- phase D: piece-waves into SBUF agg -----------
            aggacc = [wide.tile([128, NR], f16, tag=f"agac{k}",
                                name=f"agac{k}") for k in range(2)]
            toffs = []
            tacc = 0
            for ch in range(NCH):
                row = []
                for g in range(NREG):
                    row.append(tacc)
                    tacc += TPG[ch][g]
                toffs.append(row)
            assert tacc == T_E

            # wave -1: self loops (h1loc only; runs during the AG window)
            for ch in range(NCH):
                self_sb = []
                for t in range(4):
                    st = selfp.tile([128, 256], f16, tag="self")
                    rt = ch * 4 + t
                    nc.sync.dma_start(
                        out=st[:], in_=h1loc[rt * 128:(rt + 1) * 128, :])
                    self_sb.append(st)
                gps = [psA.tile([128, 512], f32, space="PSUM", tag="psA",
                                name=f"sgps{k}") for k in range(2)]
                for t in range(4):
                    for k in range(2):
                        nc.tensor.matmul(
                            gps[k][:, t * 128:(t + 1) * 128],
                            lhsT=self_sb[t][:, k * 128:(k + 1) * 128],
                            rhs=ident16[:], start=(t == 0), stop=(t == 3))
                for k in range(2):
                    nc.vector.tensor_copy(
                        out=aggacc[k][:, ch * 512:(ch + 1) * 512],
                        in_=gps[k][:])

            # waves 0..NREG-1: per AG piece, gather + scatter + accumulate
            for g in range(NREG):
                for ch in range(NCH):
                    tg = TPG[ch][g]
                    if tg == 0:
                        continue
                    toff = toffs[ch][g]
                    mt = msgp.tile([128, TPGMAX, 256], f16, tag="msg",
                                   name="msgt")
                    nc.gpsimd.dma_gather(
                        out_ap=mt[:, :tg, :],
                        in_ap=h1flat[g][:, :],
                        idxs_ap=idx_sb[:, toff * 8:(toff + tg) * 8],
                        num_idxs=tg * 128, num_idxs_reg=tg * 128,
                        elem_size=256)
                    gps = [psA.tile([128, 512], f32, space="PSUM", tag="psA",
                                    name=f"gps{k}") for k in range(2)]
                    for t in range(tg):
                        ohe = ohp.tile([128, 512], f16, tag="ohe")
                        nc.vector.tensor_scalar(
                            out=ohe[:], in0=iota_f[:],
                            scalar1=dsto_sb[:, toff + t:toff + t + 1],
                            scalar2=None, op0=OP.is_equal)
                        for k in range(2):
                            nc.tensor.matmul(
                                gps[k][:],
                                lhsT=mt[:, t, k * 128:(k + 1) * 128],
                                rhs=ohe[:], start=(t == 0), stop=(t == tg - 1))
                    for k in range(2):
                        nc.vector.tensor_tensor(
                            out=aggacc[k][:, ch * 512:(ch + 1) * 512],
                            in0=aggacc[k][:, ch * 512:(ch + 1) * 512],
                            in1=gps[k][:], op=OP.add)

            # MLP + eager pooling per chunk
            for ch in range(NCH):
                mid_sb = []
                for m in range(4):
                    mp = psB.tile([128, 512], f32, space="PSUM", tag="psB")
                    for k in range(2):
                        nc.tensor.matmul(
                            mp[:], lhsT=w1_sb[k][:, m * 128:(m + 1) * 128],
                            rhs=aggacc[k][:, ch * 512:(ch + 1) * 512],
                            start=(k == 0), stop=False)
                    nc.tensor.matmul(
                        mp[:], lhsT=ew1_sb[:, m * 128:(m + 1) * 128],
                        rhs=cntE_sb[:, ch * 512:(ch + 1) * 512],
                        start=False, stop=True)
                    ms = midp.tile([128, 512], f16, tag="mid")
                    nc.scalar.activation(out=ms[:], in_=mp[:], func=AF.Relu,
                                         bias=b1_sb[1][m][:])
                    mid_sb.append(ms)
                h2_sb = []
                for r in range(4):
                    hp = psC.tile([128, 256], f32, space="PSUM", tag="psC")
                    for k in range(4):
                        nc.tensor.matmul(
                            hp[:], lhsT=mid_sb[k][:, r * 128:(r + 1) * 128],
                            rhs=w2_sb[1][k][:], start=(k == 0), stop=False)
                    nc.tensor.matmul(hp[:], lhsT=ones_sb[:], rhs=b2_sb[1][:],
                                     start=False, stop=True)
                    hs = h2p.tile([128, 256], f16, tag="h2")
                    nc.scalar.activation(out=hs[:], in_=hp[:], func=AF.Relu)
                    h2_sb.append(hs)
                pps = [psA.tile([128, 512], f32, space="PSUM", tag="psA",
                                name=f"pps{k}") for k in range(2)]
                pohs = []
                for t in range(4):
                    poh = ohp.tile([128, 512], f16, tag="ohe", name="poh")
                    nc.vector.tensor_scalar(
                        out=poh[:], in0=iota_f[:],
                        scalar1=poff_sb[:, ch * 4 + t: ch * 4 + t + 1],
                        scalar2=None, op0=OP.is_equal)
                    pohs.append(poh)
                for k in range(2):
                    for t in range(4):
                        nc.tensor.matmul(
                            pps[k][:],
                            lhsT=h2_sb[t][:, k * 128:(k + 1) * 128],
                            rhs=pohs[t][:], start=(t == 0), stop=(t == 3))
                b = PBASE[ch]
                for k in range(2):
                    nc.vector.tensor_tensor(
                        out=cat_sb[2 + k][:, b:b + 512],
                        in0=cat_sb[2 + k][:, b:b + 512],
                        in1=pps[k][:], op=OP.add)

            # ---------------- phase F: BN1 -> proj -> BN2 -> out -----------
            def stats(tiles, n_real, loc, glob, nt_):
                npiece = 4
                psz = n_real // npiece
                assert psz * npiece == n_real
                for t in range(nt_):
                    s_sb = statp.tile([128, 2], f32, tag="stat")
                    nc.vector.tensor_reduce(
                        out=s_sb[:, 0:1], in_=tiles[t][:, 0:n_real],
                        axis=mybir.AxisListType.X, op=OP.add)
                    sp = statp.tile([128, npiece], f32, tag="sqparts")
                    for p in range(npiece):
                        sq = work.tile([128, psz], f32, tag="sqtmp")
                        nc.vector.tensor_tensor(
                            out=sq[:], in0=tiles[t][:, p * psz:(p + 1) * psz],
                            in1=tiles[t][:, p * psz:(p + 1) * psz], op=OP.mult)
                        nc.vector.tensor_reduce(
                            out=sp[:, p:p + 1], in_=sq[:],
                            axis=mybir.AxisListType.X, op=OP.add)
                    nc.vector.tensor_reduce(
                        out=s_sb[:, 1:2], in_=sp[:],
                        axis=mybir.AxisListType.X, op=OP.add)
                    nc.sync.dma_start(out=loc[t * 128:(t + 1) * 128, :],
                                      in_=s_sb[:])
                nc.gpsimd.collective_compute(
                    "AllReduce", OP.add,
                    replica_groups=[list(range(NCORE))],
                    ins=[loc[:].opt()], outs=[glob[:].opt()])
                outs = []
                for t in range(nt_):
                    g_sb = statp.tile([128, 2], f32, tag="gstat")
                    nc.sync.dma_start(out=g_sb[:],
                                      in_=glob[t * 128:(t + 1) * 128, :])
                    outs.append(g_sb)
                return outs

            def scale_bias(g_sb, gam, bet):
                mu = work.tile([128, 1], f32, tag="mu")
                nc.vector.tensor_scalar_mul(mu[:], g_sb[:, 0:1], 1.0 / N)
                var = work.tile([128, 1], f32, tag="var")
                nc.vector.tensor_scalar_mul(var[:], g_sb[:, 1:2], 1.0 / N)
                musq = work.tile([128, 1], f32, tag="musq")
                nc.vector.tensor_tensor(out=musq[:], in0=mu[:], in1=mu[:],
                                        op=OP.mult)
                nc.vector.tensor_tensor(out=var[:], in0=var[:], in1=musq[:],
                                        op=OP.subtract)
                sd = work.tile([128, 1], f32, tag="sd")
                nc.scalar.activation(out=sd[:], in_=var[:], func=AF.Sqrt,
                                     bias=eps_sb[:, 0:1])
                rstd = work.tile([128, 1], f32, tag="rstd")
                nc.vector.reciprocal(rstd[:], sd[:])
                sc = work.tile([128, 1], f32, tag="sc")
                nc.vector.tensor_tensor(out=sc[:], in0=rstd[:], in1=gam[:],
                                        op=OP.mult)
                bi = work.tile([128, 1], f32, tag="bi")
                nc.vector.tensor_tensor(out=bi[:], in0=mu[:], in1=sc[:],
                                        op=OP.mult)
                nc.vector.tensor_tensor(out=bi[:], in0=bet[:], in1=bi[:],
                                        op=OP.subtract)
                return sc, bi

            g1 = stats(cat_sb, CPC, st1loc, st1glob, 4)
            bn_sb = []
            for t in range(4):
                sc, bi = scale_bias(g1[t], bng_sb[t], bnb_sb[t])
                bt = wide.tile([128, CPAD], f16,
                               tag=(f"bn{t}" if t < 2 else f"agac{t - 2}"))
                nc.vector.tensor_scalar(
                    out=bt[:], in0=cat_sb[t][:], scalar1=sc[:, 0:1],
                    scalar2=bi[:, 0:1], op0=OP.mult, op1=OP.add)
                bn_sb.append(bt)

            out2_sb = [wide.tile([128, CPAD], f32, tag=f"cat{c2}",
                                 name=f"o2sb{c2}") for c2 in range(2)]
            for w in range(CPAD // 512):
                for c2 in range(2):
                    pp = psA.tile([128, 512], f32, space="PSUM", tag="psA")
                    for k in range(4):
                        nc.tensor.matmul(
                            pp[:],
                            lhsT=wp_sb[k][:, c2 * 128:(c2 + 1) * 128],
                            rhs=bn_sb[k][:, w * 512:(w + 1) * 512],
                            start=(k == 0), stop=(k == 3))
                    nc.vector.tensor_scalar(
                        out=out2_sb[c2][:, w * 512:(w + 1) * 512], in0=pp[:],
                        scalar1=bp_sb[c2][:, 0:1], scalar2=None, op0=OP.add)

            g2 = stats(out2_sb, CPC, st2loc, st2glob, 2)
            for c2 in range(2):
                sc, bi = scale_bias(g2[c2], ng_sb[c2], nb_sb[c2])
                nc.vector.tensor_scalar(
                    out=out2_sb[c2][:], in0=out2_sb[c2][:], scalar1=sc[:, 0:1],
                    scalar2=bi[:, 0:1], op0=OP.mult, op1=OP.add)

            for w in range(NBLK):
                os_ = work.tile([128, 256], f32, tag="outrm")
                for c2 in range(2):
                    tp = psC.tile([128, 256], f32, space="PSUM", tag="psC")
                    nc.tensor.transpose(
                        out=tp[:, 0:128],
                        in_=out2_sb[c2][:, w * 128:(w + 1) * 128],
                        identity=ident32[:])
                    nc.vector.tensor_copy(
                        out=os_[:, c2 * 128:(c2 + 1) * 128], in_=tp[:, 0:128])
                nc.sync.dma_start(out=out[w * 128:(w + 1) * 128, :],
                                  in_=os_[:])

    nc.compile()
    return nc


_CACHE = {}


def kernel(**inputs):
    from concourse.bass_utils import run_bass_kernel_spmd

    per_core, meta = _preprocess(inputs)
    wm = _weight_maps(inputs)

    key = (meta["NR"], meta["T_E"],
           tuple(tuple(r) for r in meta["TPG"]), tuple(meta["pbase"]))
    if key not in _CACHE:
        _CACHE[key] = _build(meta)
    nc = _CACHE[key]

    in_maps = []
    for c in range(NCORE):
        m = dict(per_core[c])
        m.update(wm)
        in_maps.append(m)

    trace = bool(int(os.environ.get("KERNEL_TRACE", "0")))
    res = run_bass_kernel_spmd(nc, in_maps, list(range(NCORE)), trace=trace)
    kernel.last_results = res

    outs = [res.results[c]["out"][:CPC] for c in range(NCORE)]
    return np.concatenate(outs, 0).astype(np.float32)
